# revision 10
# baseline (speedup 1.0000x reference)
"""Trainium2 Bass kernel for nn_Attention (b=8, c=256, heads=4, dh=32, n=48*48).

Sharding: batch across 8 cores (attention independent per batch item);
qkv/out projection weights replicated (host pre-transposed/cast).

Per-core plan (one batch item, x_b [256, 2304], all fp16 inputs):
  1. QK projection       q,k [128(h*d), n] fp16                   (PE)
  2. V^T projection      vaug fp16 [vT_h (32) | ones (32)] tiles   (PE)
  3. Heads processed in PAIRS (row-tiled PE strips 32h):
     per j-tile: scores S^T[j,i] for both heads into one combined
     PSUM tile [128, 2, blk]                                       (PE)
     -> single wide exp on ScalarE (scale=dh^-0.5 in the ACT affine,
        no max subtraction: scores ~ N(0,1))                       (ACT)
     -> PV immediately (col-tiled pairs across the two heads) with
        M=64 stationary [vT|ones]: rows +0:32 = O^T, +32:64 = softmax
        denominator; accumulated over j-tiles in PSUM               (PE)
  4. normalize: DVE tensor_tensor divide -> onorm (f32r)           (DVE)
  5. y = w_out(f32r) @ onorm + b_out -> DMA out                    (PE/DVE)

ScalarE exp (4*2304^2 elems/core @ ~1.2GHz) ~= 170us is the target
roofline; PE (assumed power-throttled to 1.2GHz) is kept just under it
via fp16 matmuls + row/col tile_position concurrency.
"""

import sys

if "/opt/trn_rl_repo" not in sys.path:
    sys.path.insert(0, "/opt/trn_rl_repo")

import numpy as np

import concourse.bacc as bacc
import concourse.tile as tile
from concourse import mybir
from concourse.bass_utils import run_bass_kernel_spmd

HEADS = 4
DH = 32
HID = HEADS * DH          # 128
C = 256                   # channels
N = 48 * 48               # 2304 tokens
SCALE = DH ** -0.5
F32 = mybir.dt.float32
F32R = mybir.dt.float32r
F16 = mybir.dt.float16
DIV = mybir.AluOpType.divide

NJT = N // 128            # 18 j-tiles
# i-blocks per head-pair; combined score tile is [128, 2, blen] fp32 in PSUM
BLOCKS = [(0, 1024), (1024, 1024), (2048, 256)]
NTILES = [(0, 512), (512, 512), (1024, 512), (1536, 512), (2048, 256)]


def _subs_of(blen):
    subs = []
    off = 0
    while off < blen:
        w = min(512, blen - off)
        subs.append((off, w))
        off += w
    return subs


def _kernel_body(tc, xd, wqkd, wvd, woutd, biasd, yd):
    nc = tc.nc
    import contextlib

    with contextlib.ExitStack() as stack:
        const = stack.enter_context(tc.tile_pool(name="const", bufs=1))
        qkp = stack.enter_context(tc.tile_pool(name="qkp", bufs=1))
        vap = stack.enter_context(tc.tile_pool(name="vap", bufs=1))
        onp = stack.enter_context(tc.tile_pool(name="onp", bufs=1))

        wqk = const.tile([128, 2, 2 * HID], F16, name="wqk")
        wv = const.tile([128, 2, HID], F16, name="wv")
        wout = const.tile([128, C], F32R, name="wout")
        bias = const.tile([128, 2], F32, name="bias")
        nc.sync.dma_start(out=wqk[:, 0, :], in_=wqkd[0:128, :])
        nc.sync.dma_start(out=wqk[:, 1, :], in_=wqkd[128:256, :])
        nc.sync.dma_start(out=wv[:, 0, :], in_=wvd[0:128, :])
        nc.sync.dma_start(out=wv[:, 1, :], in_=wvd[128:256, :])
        nc.sync.dma_start(out=wout[:, :], in_=woutd[:, :])
        nc.sync.dma_start(out=bias[:, :], in_=biasd[:, :])

        q = qkp.tile([128, N], F16, name="q")
        k = qkp.tile([128, N], F16, name="k")
        # PV stationary: [vT_h (32 cols) | ones (32 cols)] per (jt, h)
        vaug = vap.tile([128, NJT, HEADS, 2 * DH], F16, name="vaug")
        onorm = onp.tile([128, N], F32, name="onorm")
        onr = onp.tile([128, N], F32R, name="onr")

        # ---------------- prologue: load x, projections ----------------
        with (
            tc.tile_pool(name="xp", bufs=1) as xp,
            tc.tile_pool(name="ppsum", bufs=4, space="PSUM") as ppsum,
        ):
            xs = xp.tile([128, 2, N], F16, name="xs")
            for cc in range(2):
                nc.sync.dma_start(
                    out=xs[:, cc, 0 : N // 2],
                    in_=xd[128 * cc : 128 * cc + 128, 0 : N // 2],
                )
                nc.sync.dma_start(
                    out=xs[:, cc, N // 2 : N],
                    in_=xd[128 * cc : 128 * cc + 128, N // 2 : N],
                )

            nc.vector.memset(vaug[:, :, :, DH : 2 * DH], 1.0)

            # q, k projection: out[m, i] = sum_c wqk[c, m] * x[c, i]
            for m in range(2):  # 0 -> q rows, 1 -> k rows
                dst = q if m == 0 else k
                for off, w in NTILES:
                    pt = ppsum.tile([128, 512], F32, name="pt", tag="pt")
                    for cc in range(2):
                        nc.tensor.matmul(
                            pt[:, 0:w],
                            wqk[:, cc, 128 * m : 128 * m + 128],
                            xs[:, cc, off : off + w],
                            start=(cc == 0),
                            stop=(cc == 1),
                        )
                    nc.vector.tensor_copy(dst[:, off : off + w], pt[:, 0:w])

            # vT projection: out[i_tile, hd] = sum_c x[c, i] * wv[c, hd]
            for nt in range(NJT):
                pv = ppsum.tile([128, HID], F32, name="pv", tag="pv")
                for cc in range(2):
                    nc.tensor.matmul(
                        pv[:, :],
                        xs[:, cc, 128 * nt : 128 * nt + 128],
                        wv[:, cc, :],
                        start=(cc == 0),
                        stop=(cc == 1),
                    )
                nc.vector.tensor_copy(
                    vaug[:, nt, :, 0:DH],
                    pv.rearrange("p (h d) -> p h d", h=HEADS),
                )

        # ---------------- main attention loop (head pairs) ----------------
        with (
            tc.tile_pool(name="esp", bufs=4) as esp,
            tc.tile_pool(name="scp", bufs=1, space="PSUM") as scp,
            tc.tile_pool(name="accp", bufs=4, space="PSUM") as accp,
            tc.tile_pool(name="recp", bufs=4) as recp,
        ):
            for hp in range(2):
                h0 = 2 * hp
                for goff, blen in BLOCKS:
                    subs = _subs_of(blen)
                    # psum pad so each head's half is bank-aligned
                    bpad = max(blen, 512)
                    accs = [
                        accp.tile([128, 512], F32, name=f"acc{si}", tag="acc")
                        for si in range(len(subs))
                    ]
                    for jt in range(NJT):
                        sc = scp.tile([128, 2, bpad], F32, name="sc", tag="sc")
                        for off, w in subs:
                            for hh in range(2):
                                h = h0 + hh
                                nc.tensor.matmul(
                                    sc[:, hh, off : off + w],
                                    k[32 * h : 32 * h + 32, 128 * jt : 128 * jt + 128],
                                    q[32 * h : 32 * h + 32, goff + off : goff + off + w],
                                    start=True,
                                    stop=True,
                                    tile_position=(32 * h, 0),
                                )
                        es = esp.tile([128, 2, blen], F16, name="es", tag="es")
                        nc.scalar.activation(
                            es[:, :, :],
                            sc[:, :, 0:blen],
                            mybir.ActivationFunctionType.Exp,
                            scale=SCALE,
                        )
                        for si, (off, w) in enumerate(subs):
                            for hh in range(2):
                                nc.tensor.matmul(
                                    accs[si][64 * hh : 64 * hh + 64, 0:w],
                                    vaug[:, jt, h0 + hh, :],
                                    es[:, hh, off : off + w],
                                    start=(jt == 0),
                                    stop=(jt == NJT - 1),
                                    tile_position=(0, 64 * hh),
                                )
                    # normalize: O * (1/denom) (denom dup'd across the 32 rows)
                    for si, (off, w) in enumerate(subs):
                        for hh in range(2):
                            h = h0 + hh
                            p0 = 64 * hh
                            rec = recp.tile([32, 512], F32, name="rec", tag="rec")
                            nc.vector.reciprocal(rec[:, 0:w], accs[si][p0 + 32 : p0 + 64, 0:w])
                            nc.vector.tensor_mul(
                                onorm[32 * h : 32 * h + 32, goff + off : goff + off + w],
                                accs[si][p0 : p0 + 32, 0:w],
                                rec[:, 0:w],
                            )

        # round onorm to f32r once for the f32r output projection
        nc.vector.tensor_copy(onr[:, :], onorm[:, :])

        # ---------------- output projection ----------------
        with (
            tc.tile_pool(name="opsum", bufs=2, space="PSUM") as opsum,
            tc.tile_pool(name="ystp", bufs=3) as ystp,
        ):
            for off, w in NTILES:
                for m in range(2):
                    yp = opsum.tile([128, 512], F32, name="yp", tag="yp")
                    nc.tensor.matmul(
                        yp[:, 0:w],
                        wout[:, 128 * m : 128 * m + 128],
                        onr[:, off : off + w],
                        start=True,
                        stop=True,
                    )
                    yst = ystp.tile([128, 512], F32, name="yst", tag="yst")
                    nc.vector.tensor_scalar_add(yst[:, 0:w], yp[:, 0:w], bias[:, m : m + 1])
                    nc.sync.dma_start(
                        out=yd[128 * m : 128 * m + 128, off : off + w], in_=yst[:, 0:w]
                    )


_CACHE = {}


def _build():
    if "nc" in _CACHE:
        return _CACHE["nc"]
    nc = bacc.Bacc("TRN2", target_bir_lowering=False, debug=False)
    xd = nc.dram_tensor("x", [C, N], F16, kind="ExternalInput")
    wqkd = nc.dram_tensor("wqk_t", [C, 2 * HID], F16, kind="ExternalInput")
    wvd = nc.dram_tensor("wv_t", [C, HID], F16, kind="ExternalInput")
    woutd = nc.dram_tensor("wout_t", [HID, C], F32R, kind="ExternalInput")
    biasd = nc.dram_tensor("bias2", [128, 2], F32, kind="ExternalInput")
    yd = nc.dram_tensor("y", [C, N], F32, kind="ExternalOutput")
    with tile.TileContext(nc) as tc:
        _kernel_body(tc, xd.ap(), wqkd.ap(), wvd.ap(), woutd.ap(), biasd.ap(), yd.ap())
    nc.compile()
    _CACHE["nc"] = nc
    return nc


def _make_in_maps(x, w_qkv, w_out, b_out):
    x = np.asarray(x, dtype=np.float32).reshape(8, C, N)
    x16 = np.ascontiguousarray(x.astype(np.float16))
    w_qkv = np.asarray(w_qkv, dtype=np.float32)
    w_out = np.asarray(w_out, dtype=np.float32)
    b_out = np.asarray(b_out, dtype=np.float32)
    wqk_t = np.ascontiguousarray(w_qkv[0 : 2 * HID].T.astype(np.float16))   # [256, 256]
    wv_t = np.ascontiguousarray(w_qkv[2 * HID : 3 * HID].T.astype(np.float16))  # [256, 128]
    wout_t = np.ascontiguousarray(w_out.T)                      # [128, 256] f32
    bias2 = np.ascontiguousarray(b_out.reshape(2, 128).T)       # [128, 2]
    return [
        {"x": x16[b], "wqk_t": wqk_t, "wv_t": wv_t, "wout_t": wout_t, "bias2": bias2}
        for b in range(8)
    ]


def kernel(x, w_qkv, w_out, b_out, _trace=False):
    nc = _build()
    in_maps = _make_in_maps(x, w_qkv, w_out, b_out)
    res = run_bass_kernel_spmd(nc, in_maps, list(range(8)), trace=_trace)
    y = np.stack([res.results[b]["y"] for b in range(8)], axis=0)
    out = y.reshape(8, C, 48, 48).astype(np.float32)
    if _trace:
        _CACHE["last_exec_time_ns"] = res.exec_time_ns
        _CACHE["last_results"] = res
    return out


# revision 14
# speedup vs baseline: 1.7452x; 1.7452x over previous
"""Trainium2 Bass kernel for nn_Attention (b=8, c=256, heads=4, dh=32, n=48*48).

Sharding: batch across 8 cores (attention independent per batch item);
qkv/out projection weights replicated (host pre-transposed/cast).

Per-core plan (one batch item, x_b [256, 2304], all fp16 inputs):
  1. QK projection       q,k [128(h*d), n] fp16                   (PE)
  2. V^T projection      vaug fp16 [vT_h (32) | ones (32)] tiles   (PE)
  3. Heads processed in PAIRS (row-tiled PE strips 32h):
     per j-tile: scores S^T[j,i] for both heads into one combined
     PSUM tile [128, 2, blk]                                       (PE)
     -> single wide exp on ScalarE (scale=dh^-0.5 in the ACT affine,
        no max subtraction: scores ~ N(0,1))                       (ACT)
     -> PV immediately (col-tiled pairs across the two heads) with
        M=64 stationary [vT|ones]: rows +0:32 = O^T, +32:64 = softmax
        denominator; accumulated over j-tiles in PSUM               (PE)
  4. normalize: DVE tensor_tensor divide -> onorm (f32r)           (DVE)
  5. y = w_out(f32r) @ onorm + b_out -> DMA out                    (PE/DVE)

ScalarE exp (4*2304^2 elems/core @ ~1.2GHz) ~= 170us is the target
roofline; PE (assumed power-throttled to 1.2GHz) is kept just under it
via fp16 matmuls + row/col tile_position concurrency.
"""

import sys

if "/opt/trn_rl_repo" not in sys.path:
    sys.path.insert(0, "/opt/trn_rl_repo")

import numpy as np

import concourse.bacc as bacc
import concourse.tile as tile
from concourse import mybir
from concourse.bass_utils import run_bass_kernel_spmd

HEADS = 4
DH = 32
HID = HEADS * DH          # 128
C = 256                   # channels
N = 48 * 48               # 2304 tokens
SCALE = DH ** -0.5
F32 = mybir.dt.float32
F32R = mybir.dt.float32r
F16 = mybir.dt.float16
DIV = mybir.AluOpType.divide

NJT = N // 128            # 18 j-tiles
# i-blocks per head-pair: 4x512 + 256. Combined score tile [128, 2, 512]
# fp32 = 2 PSUM banks: each head's matmul output owns a full bank (two
# concurrent row-tiled matmuls must not share a bank), while ACT exp reads
# both heads in a single call.
BLOCKS = [(0, 512), (512, 512), (1024, 512), (1536, 512), (2048, 256)]
NTILES = [(0, 512), (512, 512), (1024, 512), (1536, 512), (2048, 256)]


def _kernel_body(tc, xd, wqkd, wvd, woutd, biasd, yd):
    nc = tc.nc
    import contextlib

    with contextlib.ExitStack() as stack:
        const = stack.enter_context(tc.tile_pool(name="const", bufs=1))
        qkp = stack.enter_context(tc.tile_pool(name="qkp", bufs=1))
        vap = stack.enter_context(tc.tile_pool(name="vap", bufs=1))
        onp = stack.enter_context(tc.tile_pool(name="onp", bufs=1))

        wqk = const.tile([128, 2, 2 * HID], F16, name="wqk")
        wv = const.tile([128, 2, HID], F16, name="wv")
        wout = const.tile([128, C], F32R, name="wout")
        bias = const.tile([128, 2], F32, name="bias")
        nc.sync.dma_start(out=wqk[:, 0, :], in_=wqkd[0:128, :])
        nc.sync.dma_start(out=wqk[:, 1, :], in_=wqkd[128:256, :])
        nc.sync.dma_start(out=wv[:, 0, :], in_=wvd[0:128, :])
        nc.sync.dma_start(out=wv[:, 1, :], in_=wvd[128:256, :])
        nc.sync.dma_start(out=wout[:, :], in_=woutd[:, :])
        nc.sync.dma_start(out=bias[:, :], in_=biasd[:, :])

        q = qkp.tile([128, N], F16, name="q")
        k = qkp.tile([128, N], F16, name="k")
        # PV stationary: [vT_h (32 cols) | ones (32 cols)] per (jt, h)
        vaug = vap.tile([128, NJT, HEADS, 2 * DH], F16, name="vaug")
        onorm = onp.tile([128, N], F32, name="onorm")
        onr = onp.tile([128, N], F32R, name="onr")

        # ---------------- prologue: load x, projections ----------------
        with (
            tc.tile_pool(name="xp", bufs=1) as xp,
            tc.tile_pool(name="ppsum", bufs=4, space="PSUM") as ppsum,
        ):
            xs = xp.tile([128, 2, N], F16, name="xs")
            for cc in range(2):
                nc.sync.dma_start(
                    out=xs[:, cc, 0 : N // 2],
                    in_=xd[128 * cc : 128 * cc + 128, 0 : N // 2],
                )
                nc.sync.dma_start(
                    out=xs[:, cc, N // 2 : N],
                    in_=xd[128 * cc : 128 * cc + 128, N // 2 : N],
                )

            nc.vector.memset(vaug[:, :, :, DH : 2 * DH], 1.0)

            # q, k projection: out[m, i] = sum_c wqk[c, m] * x[c, i]
            for m in range(2):  # 0 -> q rows, 1 -> k rows
                dst = q if m == 0 else k
                for off, w in NTILES:
                    pt = ppsum.tile([128, 512], F32, name="pt", tag="pt")
                    for cc in range(2):
                        nc.tensor.matmul(
                            pt[:, 0:w],
                            wqk[:, cc, 128 * m : 128 * m + 128],
                            xs[:, cc, off : off + w],
                            start=(cc == 0),
                            stop=(cc == 1),
                        )
                    nc.vector.tensor_copy(dst[:, off : off + w], pt[:, 0:w])

            # vT projection: out[i_tile, hd] = sum_c x[c, i] * wv[c, hd]
            for nt in range(NJT):
                pv = ppsum.tile([128, HID], F32, name="pv", tag="pv")
                for cc in range(2):
                    nc.tensor.matmul(
                        pv[:, :],
                        xs[:, cc, 128 * nt : 128 * nt + 128],
                        wv[:, cc, :],
                        start=(cc == 0),
                        stop=(cc == 1),
                    )
                nc.vector.tensor_copy(
                    vaug[:, nt, :, 0:DH],
                    pv.rearrange("p (h d) -> p h d", h=HEADS),
                )

        # ---------------- main attention loop (head pairs) ----------------
        with (
            tc.tile_pool(name="esp", bufs=4) as esp,
            tc.tile_pool(name="scp", bufs=3, space="PSUM") as scp,
            tc.tile_pool(name="accp", bufs=2, space="PSUM") as accp,
            tc.tile_pool(name="recp", bufs=4) as recp,
        ):
            for hp in range(2):
                h0 = 2 * hp
                for goff, w in BLOCKS:
                    acc = accp.tile([128, 512], F32, name="acc", tag="acc")
                    for jt in range(NJT):
                        sc = scp.tile([128, 2, 512], F32, name="sc", tag="sc")
                        for hh in range(2):
                            h = h0 + hh
                            nc.tensor.matmul(
                                sc[:, hh, 0:w],
                                k[32 * h : 32 * h + 32, 128 * jt : 128 * jt + 128],
                                q[32 * h : 32 * h + 32, goff : goff + w],
                                start=True,
                                stop=True,
                                tile_position=(32 * h, 0),
                            )
                        es = esp.tile([128, 2, 512], F16, name="es", tag="es")
                        nc.scalar.activation(
                            es[:, :, 0:w],
                            sc[:, :, 0:w],
                            mybir.ActivationFunctionType.Exp,
                            scale=SCALE,
                        )
                        for hh in range(2):
                            nc.tensor.matmul(
                                acc[64 * hh : 64 * hh + 64, 0:w],
                                vaug[:, jt, h0 + hh, :],
                                es[:, hh, 0:w],
                                start=(jt == 0),
                                stop=(jt == NJT - 1),
                                tile_position=(0, 64 * hh),
                            )
                    # normalize: O * (1/denom) (denom dup'd across the 32 rows)
                    for hh in range(2):
                        h = h0 + hh
                        p0 = 64 * hh
                        rec = recp.tile([32, 512], F32, name="rec", tag="rec")
                        nc.vector.reciprocal(rec[:, 0:w], acc[p0 + 32 : p0 + 64, 0:w])
                        nc.vector.tensor_mul(
                            onorm[32 * h : 32 * h + 32, goff : goff + w],
                            acc[p0 : p0 + 32, 0:w],
                            rec[:, 0:w],
                        )

        # round onorm to f32r once for the f32r output projection
        nc.vector.tensor_copy(onr[:, :], onorm[:, :])

        # ---------------- output projection ----------------
        with (
            tc.tile_pool(name="opsum", bufs=2, space="PSUM") as opsum,
            tc.tile_pool(name="ystp", bufs=3) as ystp,
        ):
            for off, w in NTILES:
                for m in range(2):
                    yp = opsum.tile([128, 512], F32, name="yp", tag="yp")
                    nc.tensor.matmul(
                        yp[:, 0:w],
                        wout[:, 128 * m : 128 * m + 128],
                        onr[:, off : off + w],
                        start=True,
                        stop=True,
                    )
                    yst = ystp.tile([128, 512], F32, name="yst", tag="yst")
                    nc.vector.tensor_scalar_add(yst[:, 0:w], yp[:, 0:w], bias[:, m : m + 1])
                    nc.sync.dma_start(
                        out=yd[128 * m : 128 * m + 128, off : off + w], in_=yst[:, 0:w]
                    )


_CACHE = {}


def _build():
    if "nc" in _CACHE:
        return _CACHE["nc"]
    nc = bacc.Bacc("TRN2", target_bir_lowering=False, debug=False)
    xd = nc.dram_tensor("x", [C, N], F16, kind="ExternalInput")
    wqkd = nc.dram_tensor("wqk_t", [C, 2 * HID], F16, kind="ExternalInput")
    wvd = nc.dram_tensor("wv_t", [C, HID], F16, kind="ExternalInput")
    woutd = nc.dram_tensor("wout_t", [HID, C], F32R, kind="ExternalInput")
    biasd = nc.dram_tensor("bias2", [128, 2], F32, kind="ExternalInput")
    yd = nc.dram_tensor("y", [C, N], F32, kind="ExternalOutput")
    with tile.TileContext(nc) as tc:
        _kernel_body(tc, xd.ap(), wqkd.ap(), wvd.ap(), woutd.ap(), biasd.ap(), yd.ap())
    nc.compile()
    _CACHE["nc"] = nc
    return nc


def _make_in_maps(x, w_qkv, w_out, b_out):
    x = np.asarray(x, dtype=np.float32).reshape(8, C, N)
    x16 = np.ascontiguousarray(x.astype(np.float16))
    w_qkv = np.asarray(w_qkv, dtype=np.float32)
    w_out = np.asarray(w_out, dtype=np.float32)
    b_out = np.asarray(b_out, dtype=np.float32)
    wqk_t = np.ascontiguousarray(w_qkv[0 : 2 * HID].T.astype(np.float16))   # [256, 256]
    wv_t = np.ascontiguousarray(w_qkv[2 * HID : 3 * HID].T.astype(np.float16))  # [256, 128]
    wout_t = np.ascontiguousarray(w_out.T)                      # [128, 256] f32
    bias2 = np.ascontiguousarray(b_out.reshape(2, 128).T)       # [128, 2]
    return [
        {"x": x16[b], "wqk_t": wqk_t, "wv_t": wv_t, "wout_t": wout_t, "bias2": bias2}
        for b in range(8)
    ]


def kernel(x, w_qkv, w_out, b_out, _trace=False):
    nc = _build()
    in_maps = _make_in_maps(x, w_qkv, w_out, b_out)
    res = run_bass_kernel_spmd(nc, in_maps, list(range(8)), trace=_trace)
    y = np.stack([res.results[b]["y"] for b in range(8)], axis=0)
    out = y.reshape(8, C, 48, 48).astype(np.float32)
    if _trace:
        _CACHE["last_exec_time_ns"] = res.exec_time_ns
        _CACHE["last_results"] = res
    return out
